# revision 30
# baseline (speedup 1.0000x reference)
"""Trainium2 Bass kernel for causal GQA self-attention (fused QKV + RoPE).

Problem: B=2, T=2048, C=2048, H=16 q-heads, KV=4 kv-heads, HD=128.
Sharding: 8 cores = (batch b, kv-group k). Each core computes the 4 q-heads
of one kv group for one batch element; outputs are disjoint slices of y.

v3 (bf16 datapath + pair-interleaved schedule):
  - All matmul inputs bf16 (fastest PE dtype on TRN2: ~216ns per 512-col
    matmul vs 227 fp32r / 259 fp16; no N<256 penalty). PSUM stays fp32.
  - Projection streams: per t-block, two 3-bank PSUM groups contract all
    of C; weight/x DMAs interleaved (first ci-slice split finer) so the PE
    starts after ~0.3MB instead of 10MB.
  - Attention pairs (tb,h) interleaved INTO the projection stages at jg
    granularity so exp work on the scalar engine (the attention-phase
    critical engine, ~690ns/chunk) starts as early as dependencies allow.
  - Per pair: diagonal chunks first (exp+tri latency hidden by the
    off-diag stream), scores/PV software-pipelined; row sums (ones-matmul)
    batched at pair end.
Output per core: unnormalized y^T [512, 2048] bf16 + row sums [16, 512]
fp32; the host divides, transposes and concatenates.
"""

import math

import ml_dtypes
import numpy as np

import concourse.bass as bass
import concourse.mybir as mybir
import concourse.tile as tile
from concourse import bacc
from concourse.bass_utils import run_bass_kernel_spmd

B, T, C = 2, 2048, 2048
H, KV, HD = 16, 4, 128
NREP = H // KV  # q heads per core
P = 128
NCORES = 8
TT = 4  # t-blocks of 512
TB = T // TT  # 512
NB = 6  # j-blocks per core: q0..q3, k, v
SCALE = 1.0 / math.sqrt(HD)

f32 = mybir.dt.float32
bf16 = mybir.dt.bfloat16
bf16_np = ml_dtypes.bfloat16

TRACE = False  # set True (with ntff shim installed) to get exec_time_ns

_cache = {}


def _build():
    if "nc" in _cache:
        return _cache["nc"]

    nc = bacc.Bacc("TRN2", target_bir_lowering=False, debug=False,
                   num_devices=NCORES)

    # xT tiled [p, tt, cq, ci, tb]: slice (tt, cq) contiguous per partition
    xT_d = nc.dram_tensor("xT", [P, TT, 4, 4, TB], bf16, kind="ExternalInput").ap()
    # wT tiled [p, cq, ci, j]: slice cq contiguous per partition
    wT_d = nc.dram_tensor("wT", [P, 4, 4, NB * P], bf16, kind="ExternalInput").ap()
    cc_d = nc.dram_tensor("CC", [P, T], bf16, kind="ExternalInput").ap()
    ss_d = nc.dram_tensor("SS2", [P, T], bf16, kind="ExternalInput").ap()
    tri_d = nc.dram_tensor("tri", [P, P], bf16, kind="ExternalInput").ap()
    ones_d = nc.dram_tensor("ones", [P, P], bf16, kind="ExternalInput").ap()
    ident_d = nc.dram_tensor("ident", [P, P], bf16, kind="ExternalInput").ap()
    yT_d = nc.dram_tensor("yT", [NREP * P, T], bf16, kind="ExternalOutput").ap()
    sums_d = nc.dram_tensor("sums", [NREP * TT, TB], f32, kind="ExternalOutput").ap()

    with tile.TileContext(nc) as tc:
        with (
            tc.tile_pool(name="wt", bufs=1) as wt_pool,
            tc.tile_pool(name="xt", bufs=8) as xt_pool,
            tc.tile_pool(name="qkvt", bufs=1) as qkv_pool,
            tc.tile_pool(name="freq", bufs=1) as freq_pool,
            tc.tile_pool(name="small", bufs=1) as small_pool,
            tc.tile_pool(name="vsb", bufs=1) as v_pool,
            tc.tile_pool(name="swp", bufs=2) as swp_pool,
            tc.tile_pool(name="ropetmp", bufs=4) as rt_pool,
            tc.tile_pool(name="expt", bufs=18) as exp_pool,
            tc.tile_pool(name="yout", bufs=2) as y_pool,
            tc.tile_pool(name="ppj", bufs=4, space="PSUM") as pp_proj,
            tc.tile_pool(name="pps", bufs=2, space="PSUM") as pp_s,
            tc.tile_pool(name="ppym", bufs=2, space="PSUM") as pp_ym,
        ):
            # ---- resident tensors ----
            wt_q = [
                wt_pool.tile([P, 4, NB * P], bf16, tag=f"wt{cq}", name=f"wt{cq}")
                for cq in range(4)
            ]
            qkvT = [
                qkv_pool.tile([P, T], bf16, tag=f"qkv{jb}", name=f"qkv{jb}")
                for jb in range(NB)
            ]
            v_sb = v_pool.tile([P, 16, P], bf16, tag="vsb", name="v_sb")
            ccs = freq_pool.tile([P, T], bf16, tag="cc", name="ccs")
            ss2 = freq_pool.tile([P, T], bf16, tag="ss", name="ss2")
            tri = small_pool.tile([P, P], bf16, tag="tri", name="tri")
            ones = small_pool.tile([P, P], bf16, tag="ones", name="ones")
            ident = small_pool.tile([P, P], bf16, tag="ident", name="ident")

            xt = {}  # (tt, cq) -> tile

            def fetch_xt(tt):
                for cq in range(4):
                    t_ = xt_pool.tile([P, 4, TB], bf16, tag="xt", name="xt")
                    nc.sync.dma_start(t_[:], xT_d[:, tt, cq])
                    xt[(tt, cq)] = t_

            # warm up the PE clock ramp during the startup DMA window with
            # throwaway matmuls on a zeroed dummy tile
            dummy = small_pool.tile([P, TB], bf16, tag="dummy", name="dummy")
            nc.vector.memset(dummy[:], 0.0)
            for _ in range(12):
                wps = pp_s.tile([P, TB], f32, tag="ps", name="warm_ps")
                nc.tensor.matmul(wps[:], dummy[:, 0:P], dummy[:],
                                 start=True, stop=True)

            # fine-grained first arrivals: the PE's first matmul needs only
            # (cq0, ci0) of the weights and x
            t_ = xt_pool.tile([P, 4, TB], bf16, tag="xt", name="xt")
            xt[(0, 0)] = t_
            nc.sync.dma_start(wt_q[0][:, 0, 0:3 * P], wT_d[:, 0, 0, 0:3 * P])
            nc.sync.dma_start(t_[:, 0], xT_d[:, 0, 0, 0])
            nc.sync.dma_start(wt_q[0][:, 0, 3 * P:], wT_d[:, 0, 0, 3 * P:])
            for ci in range(1, 4):
                nc.sync.dma_start(wt_q[0][:, ci], wT_d[:, 0, ci])
                nc.sync.dma_start(t_[:, ci], xT_d[:, 0, 0, ci])
            for cq in range(1, 4):
                nc.sync.dma_start(wt_q[cq][:], wT_d[:, cq])
                t_ = xt_pool.tile([P, 4, TB], bf16, tag="xt", name="xt")
                nc.sync.dma_start(t_[:], xT_d[:, 0, cq])
                xt[(0, cq)] = t_
            fetch_xt(1)
            for _tile, _src in ((ccs, cc_d), (ss2, ss_d), (tri, tri_d),
                                (ones, ones_d), (ident, ident_d)):
                nc.sync.dma_start(_tile[:], _src[:])

            def proj_group(tt, jg, jbs=None, ps=None):
                if jg == 0 and 1 < tt + 1 < TT:
                    fetch_xt(tt + 1)
                tsl = slice(tt * TB, (tt + 1) * TB)
                if jbs is None:
                    jbs = [3 * jg, 3 * jg + 1, 3 * jg + 2]
                if ps is None:
                    ps = [
                        pp_proj.tile([P, TB], f32, tag="pj", name="proj_ps")
                        for _ in jbs
                    ]
                for cq in range(4):
                    for ci in range(4):
                        for k, jb in enumerate(jbs):
                            nc.tensor.matmul(
                                ps[k][:],
                                wt_q[cq][:, ci, jb * P:(jb + 1) * P],
                                xt[(tt, cq)][:, ci, :],
                                start=(cq == 0 and ci == 0),
                                stop=(cq == 3 and ci == 3),
                            )
                for k, jb in enumerate(jbs):
                    nc.vector.tensor_copy(qkvT[jb][:, tsl], ps[k][:])

            def v_transpose(tt):
                for i in range(4):
                    sc = 4 * tt + i
                    trp = pp_s.tile([P, P], bf16, tag="ps", name="trp")
                    nc.tensor.transpose(
                        trp[:], qkvT[5][:, sc * P:(sc + 1) * P], ident[:]
                    )
                    nc.vector.tensor_copy(v_sb[:, sc, :], trp[:])

            def rope_stage(tt):
                tsl = slice(tt * TB, (tt + 1) * TB)
                for jb in range(5):
                    swp = swp_pool.tile([P, TB], bf16, tag="swp", name="swp")
                    nc.sync.dma_start(swp[0:64, :], qkvT[jb][64:128, tsl])
                    nc.sync.dma_start(swp[64:128, :], qkvT[jb][0:64, tsl])
                    ta = rt_pool.tile([P, TB], bf16, tag="ta", name="ta")
                    tb_ = rt_pool.tile([P, TB], bf16, tag="tb", name="tb")
                    nc.vector.tensor_tensor(
                        ta[:], qkvT[jb][:, tsl], ccs[:, tsl], mybir.AluOpType.mult
                    )
                    nc.vector.tensor_tensor(
                        tb_[:], swp[:], ss2[:, tsl], mybir.AluOpType.mult
                    )
                    nc.vector.tensor_tensor(
                        qkvT[jb][:, tsl], ta[:], tb_[:], mybir.AluOpType.add
                    )

            def attn_pair(tb, h):
                nsc = 4 * (tb + 1)
                psum_y = pp_ym.tile([P, TB], f32, tag="ym", name="psum_y")
                psum_sum = pp_ym.tile([P, TB], f32, tag="ym", name="psum_sum")
                # diagonal chunks first: their exp+tri latency hides behind
                # the off-diagonal stream
                seq = [4 * tb + r for r in range(4)] + list(range(4 * tb))
                exps = {}

                def s_and_exp(sc):
                    col0 = (sc - 4 * tb) * P if sc >= 4 * tb else 0
                    ps = pp_s.tile([P, TB], f32, tag="ps", name="psum_s")
                    nc.tensor.matmul(
                        ps[:, col0:],
                        qkvT[4][:, sc * P:(sc + 1) * P],
                        qkvT[h][:, tb * TB + col0:(tb + 1) * TB],
                        start=True,
                        stop=True,
                    )
                    ex = exp_pool.tile([P, TB], bf16, tag="ex", name="expt")
                    nc.scalar.activation(
                        ex[:, col0:], ps[:, col0:],
                        mybir.ActivationFunctionType.Exp, scale=SCALE,
                    )
                    if sc >= 4 * tb:
                        nc.vector.tensor_tensor(
                            ex[:, col0:col0 + P], ex[:, col0:col0 + P],
                            tri[:], mybir.AluOpType.mult,
                        )
                    exps[sc] = (ex, col0)

                def pv(sc, first, last):
                    ex, col0 = exps[sc]
                    nc.tensor.matmul(
                        psum_y[:, col0:], v_sb[:, sc, :], ex[:, col0:],
                        start=first, stop=last,
                    )

                pend = []
                for sc in seq:
                    s_and_exp(sc)
                    pend.append(sc)
                    if len(pend) == 2:
                        psc = pend.pop(0)
                        pv(psc, psc == seq[0], psc == seq[-1])
                psc = pend.pop(0)
                pv(psc, psc == seq[0], psc == seq[-1])

                # batched row sums at pair end
                for i, sc in enumerate(seq):
                    ex, col0 = exps[sc]
                    nc.tensor.matmul(
                        psum_sum[:, col0:], ones[:], ex[:, col0:],
                        start=(i == 0), stop=(i == nsc - 1),
                    )

                y_sb = y_pool.tile([P, TB], bf16, tag="ysb", name="y_sb")
                nc.vector.tensor_copy(y_sb[:], psum_y[:])
                nc.sync.dma_start(
                    yT_d[h * P:(h + 1) * P, tb * TB:(tb + 1) * TB], y_sb[:]
                )
                sums_sb = y_pool.tile([1, TB], f32, tag="sums", name="sums_sb")
                nc.scalar.copy(sums_sb[:], psum_sum[0:1, :])
                nc.sync.dma_start(
                    sums_d[h * TT + tb:h * TT + tb + 1, :], sums_sb[0:1, :]
                )

            # ---- schedule: attention pairs interleaved into proj stages so
            # exp (scalar engine) starts as early as dependencies allow ----
            # P0 runs as ONE 6-wide group (borrowing the idle attention psum
            # banks) so the early weight+x DMA feed rate is halved
            ps6 = [
                pp_proj.tile([P, TB], f32, tag="pj", name="proj_ps")
                for _ in range(4)
            ] + [
                pp_s.tile([P, TB], f32, tag="ps", name="proj_ps_s")
                for _ in range(2)
            ]
            proj_group(0, 0, jbs=list(range(6)), ps=ps6)
            rope_stage(0)
            proj_group(1, 0)
            v_transpose(0)
            attn_pair(0, 0)
            attn_pair(0, 1)
            proj_group(1, 1)
            rope_stage(1)
            attn_pair(0, 2)
            attn_pair(0, 3)
            proj_group(2, 0)
            v_transpose(1)
            attn_pair(1, 0)
            attn_pair(1, 1)
            proj_group(2, 1)
            rope_stage(2)
            attn_pair(1, 2)
            attn_pair(1, 3)
            proj_group(3, 0)
            v_transpose(2)
            attn_pair(2, 0)
            attn_pair(2, 1)
            proj_group(3, 1)
            rope_stage(3)
            v_transpose(3)
            attn_pair(2, 2)
            attn_pair(2, 3)
            for h in range(NREP):
                attn_pair(3, h)

    nc.compile()
    _cache["nc"] = nc
    return nc


def _host_prep(x, w_qkv, freqs_cos, freqs_sin):
    """Build per-core input maps (numpy, cheap)."""
    x = np.asarray(x, dtype=np.float32)
    w_qkv = np.asarray(w_qkv, dtype=np.float32)
    freqs_cos = np.asarray(freqs_cos, dtype=np.float32)
    freqs_sin = np.asarray(freqs_sin, dtype=np.float32)

    perm = np.concatenate([np.arange(0, HD, 2), np.arange(1, HD, 2)])

    xTs = []
    for b in range(B):
        # [p, tt, cq, ci, tb] from x[b] [t, c]
        xt = np.ascontiguousarray(
            x[b].reshape(TT, TB, 16, P).transpose(3, 0, 2, 1)
            .reshape(P, TT, 4, 4, TB).astype(bf16_np)
        )
        xTs.append(xt)

    cosT = freqs_cos.T  # [64, T]
    sinT = freqs_sin.T
    CCh = np.ascontiguousarray(
        np.concatenate([cosT, cosT], axis=0).astype(bf16_np))
    SS2 = np.ascontiguousarray(
        np.concatenate([-sinT, sinT], axis=0).astype(bf16_np))
    tri = np.triu(np.ones((P, P), dtype=bf16_np))
    ones = np.ones((P, P), dtype=bf16_np)
    ident = np.eye(P, dtype=bf16_np)

    in_maps = []
    for core in range(NCORES):
        b, kv = divmod(core, KV)
        blocks = []
        for r in range(NREP):
            hrow = (kv * NREP + r) * HD
            blocks.append(w_qkv[hrow:hrow + HD][perm])
        blocks.append(w_qkv[H * HD + kv * HD:H * HD + (kv + 1) * HD][perm])
        blocks.append(
            w_qkv[(H + KV) * HD + kv * HD:(H + KV) * HD + (kv + 1) * HD]
        )
        w_shard = np.concatenate(blocks, axis=0)  # [768, C]
        wT = np.ascontiguousarray(
            w_shard.T.reshape(4, 4, P, NB * P).transpose(2, 0, 1, 3)
            .astype(bf16_np)
        )
        in_maps.append({
            "xT": xTs[b],
            "wT": wT,
            "CC": CCh,
            "SS2": SS2,
            "tri": tri,
            "ones": ones,
            "ident": ident,
        })
    return in_maps


def kernel(x, w_qkv, freqs_cos, freqs_sin):
    nc = _build()
    in_maps = _host_prep(x, w_qkv, freqs_cos, freqs_sin)
    res = run_bass_kernel_spmd(nc, in_maps, list(range(NCORES)), trace=TRACE)
    _cache["last_res"] = res

    y = np.empty((B, T, C), dtype=np.float32)
    for core in range(NCORES):
        b, kv = divmod(core, KV)
        yT = np.asarray(res.results[core]["yT"]).astype(np.float32)
        sums = np.asarray(res.results[core]["sums"]).reshape(NREP, T)
        yT = yT.reshape(NREP, P, T) / sums[:, None, :]
        y[b, :, kv * NREP * HD:(kv + 1) * NREP * HD] = (
            yT.reshape(NREP * P, T).T
        )
    return y
